# revision 4
# baseline (speedup 1.0000x reference)
"""BiLSTM-CRF loss kernel for Trainium2 (8 NeuronCores, SPMD data parallel).

Per core (batch slice of 4 sequences, full S=512):
  - embedding gather (indirect DMA) from the replicated 32000x300 fp32 table
  - input projection for both LSTM directions (bf16 matmuls, bias folded in
    via a constant-one input row)
  - both LSTM recurrences (For_i hardware loop over t; TensorE recurrence
    matmuls + ACT sigmoid/tanh + DVE elementwise)
  - tag projection + CRF forward recursion fused in a second For_i loop
    (exp-matmul trick: cand = ln(exp(prev - m) @ exp(trans + b_tag)) + m + feat)
  - returns feats (tag-major, no b_tag) and per-sequence forward scores

Host: gold score (pure indexing) + final mean. Constants (embedding table,
weights) are committed to device memory once and reused across calls; the
jitted shard_map executable is likewise built once per process.
"""
import sys

sys.path.insert(0, "/opt/trn_rl_repo")

import numpy as np
import ml_dtypes

import concourse.bass as bass
import concourse.mybir as mybir
import concourse.tile as tile
from concourse import bacc
from concourse.bass import ds
from concourse.masks import make_identity

B, S, V, E, HD, T = 32, 512, 32000, 300, 256, 11
NCORES = 8
BL = B // NCORES          # 4 sequences per core
EP = 384                  # E padded: cols 0-299 data, 300 = 1.0 (bias row), rest 0
G = 2048                  # gate columns: 0-1023 fwd (i,f,g,o), 1024-2047 bwd
START_TAG, STOP_TAG = 9, 10
TRANS_CLIP = -30.0

F32 = mybir.dt.float32
BF16 = mybir.dt.bfloat16
I32 = mybir.dt.int32
AF = mybir.ActivationFunctionType
OP = mybir.AluOpType
AX = mybir.AxisListType


def build(s=S, run_lstm=True, run_crf=True, crf_unroll=8, crf_db=False):
    """Build the per-core bass kernel (parameterized by sequence length for sim).

    run_lstm/run_crf=False build timing-variant kernels with those phases
    skipped (outputs are garbage); used only for profiling attribution.
    crf_unroll: max_unroll of the CRF For_i loop.
    crf_db: double-buffer the CRF psum tiles (ft/q) by borrowing the
    proj-phase psum pools (2 slots each) instead of dedicated 1-slot pools.
    """
    nt_seq = s // 128          # token tiles per sequence
    nt = BL * nt_seq           # token tiles per core
    nc = bacc.Bacc()

    tok = nc.dram_tensor("tok", [128, nt], I32, kind="ExternalInput")
    mask = nc.dram_tensor("mask", [BL, s], F32, kind="ExternalInput")
    labf = nc.dram_tensor("labf", [1, BL, s], F32, kind="ExternalInput")
    emb = nc.dram_tensor("emb", [V, E], F32, kind="ExternalInput")
    wcat = nc.dram_tensor("wcat", [EP, G], BF16, kind="ExternalInput")
    whh = nc.dram_tensor("whh", [HD, G], BF16, kind="ExternalInput")
    wtag = nc.dram_tensor("wtag", [4 * 128, T], BF16, kind="ExternalInput")
    ecrf = nc.dram_tensor("ecrf", [T, T], F32, kind="ExternalInput")
    stv = nc.dram_tensor("stv", [T, 1], F32, kind="ExternalInput")
    tgi = nc.dram_tensor("tgi", [T, 1], F32, kind="ExternalInput")

    em_out = nc.dram_tensor("em_out", [1, BL], F32, kind="ExternalOutput")
    fs_out = nc.dram_tensor("fs_out", [BL, 1], F32, kind="ExternalOutput")

    with tile.TileContext(nc) as tc:
        with (
            tc.tile_pool(name="persist", bufs=1) as pp,
            tc.tile_pool(name="stage", bufs=2) as sp,
            tc.tile_pool(name="ps_tr", bufs=2, space="PSUM") as ps_tr,
            tc.tile_pool(name="ps_pj", bufs=2, space="PSUM") as ps_pj,
            tc.tile_pool(name="ps_g", bufs=1, space="PSUM") as ps_g,
            tc.tile_pool(name="ps_c", bufs=1, space="PSUM") as ps_c,
        ):
            # ---- constants into SBUF ----
            idx = pp.tile([128, nt], I32)
            nc.sync.dma_start(idx[:], tok[:])
            mask_sb = pp.tile([BL, s], F32)
            nc.sync.dma_start(mask_sb[:], mask[:])
            wcat_sb = pp.tile([128, EP // 128, G], BF16)
            nc.sync.dma_start(wcat_sb[:], wcat.rearrange("(k p) n -> p k n", p=128))
            whh_sb = pp.tile([128, HD // 128, G], BF16)
            nc.sync.dma_start(whh_sb[:], whh.rearrange("(k p) n -> p k n", p=128))
            wtag_sb = pp.tile([128, 4, T], BF16)
            nc.sync.dma_start(wtag_sb[:], wtag.rearrange("(k p) n -> p k n", p=128))
            e_sb = pp.tile([T, T], F32)
            nc.sync.dma_start(e_sb[:], ecrf[:])
            stv_sb = pp.tile([T, 1], F32)
            nc.sync.dma_start(stv_sb[:], stv[:])
            tgi_sb = pp.tile([T, 1], F32)
            nc.sync.dma_start(tgi_sb[:], tgi[:])
            labf_sb = pp.tile([1, BL, s], F32)
            nc.sync.dma_start(labf_sb[:], labf[:])
            ident = pp.tile([128, 128], F32)
            make_identity(nc, ident[:])

            # ---- persistent state ----
            xwt = pp.tile([128, 16, BL, s], BF16)          # gate-major input proj
            hf = pp.tile([128, 2, BL, s + 1], BF16)        # h_f[t] at slot t+1
            hb = pp.tile([128, 2, BL, s + 1], BF16)        # h_b[t] at slot t
            feats_sb = pp.tile([128, BL, s], F32)          # rows 0..10 valid
            nc.vector.memset(hf[:, :, :, 0:1], 0.0)
            nc.vector.memset(hb[:, :, :, s : s + 1], 0.0)

            # ================= P2: gather + transpose + input projection ======
            for j in range(BL):  # chunk = one sequence (s tokens = nt_seq tiles)
                embq = sp.tile([128, nt_seq, EP], F32, tag="embq")
                nc.vector.memset(embq[:, :, E:], 0.0)
                nc.vector.memset(embq[:, :, E : E + 1], 1.0)  # bias row
                for i in range(nt_seq):
                    nc.gpsimd.indirect_dma_start(
                        out=embq[:, i, :E],
                        out_offset=None,
                        in_=emb[:, :],
                        in_offset=bass.IndirectOffsetOnAxis(
                            ap=idx[:, j * nt_seq + i : j * nt_seq + i + 1], axis=0
                        ),
                    )
                xtc = sp.tile([128, EP // 128, s], BF16, tag="xtc")
                for i in range(nt_seq):
                    for k in range(EP // 128):
                        trp = ps_tr.tile([128, 128], F32, tag="tr")
                        nc.tensor.transpose(
                            trp[:], embq[:, i, k * 128 : (k + 1) * 128], ident[:]
                        )
                        nc.vector.tensor_copy(
                            xtc[:, k, i * 128 : (i + 1) * 128], trp[:]
                        )
                for m in range(16):
                    pj = ps_pj.tile([128, s], F32, tag="pj")
                    for k in range(EP // 128):
                        nc.tensor.matmul(
                            pj[:],
                            lhsT=wcat_sb[:, k, m * 128 : (m + 1) * 128],
                            rhs=xtc[:, k, :],
                            start=(k == 0),
                            stop=(k == EP // 128 - 1),
                        )
                    nc.vector.tensor_copy(xwt[:, m, j, :], pj[:])

            # ================= P3: both LSTM scans =============================
            scratch = {}
            for d in ("f", "b"):
                shapes = {"c": 2, "g": 8, "sif": 4, "tg": 2, "so": 2, "t1": 2, "tc": 2}
                scratch[d] = {
                    k: pp.tile([128, n, BL], F32, name=f"{k}_{d}", tag=f"{k}_{d}")
                    for k, n in shapes.items()
                }
                nc.vector.memset(scratch[d]["c"][:], 0.0)

            def lstm_step(t):
                for d, hst, rd, wr, xcol, moff in (
                    ("f", hf, t, t + 1, t, 0),
                    ("b", hb, s - t, s - 1 - t, s - 1 - t, 8),
                ):
                    sc = scratch[d]
                    gps = ps_g.tile([128, 8, BL], F32, tag=f"gps_{d}")
                    for m in range(8):
                        for k in range(2):
                            nc.tensor.matmul(
                                gps[:, m, :],
                                lhsT=whh_sb[:, k, (moff + m) * 128 : (moff + m + 1) * 128],
                                rhs=hst[:, k, :, ds(rd, 1)].squeeze(2),
                                start=(k == 0),
                                stop=(k == 1),
                            )
                    nc.vector.tensor_tensor(
                        out=sc["g"][:], in0=gps[:],
                        in1=xwt[:, moff : moff + 8, :, ds(xcol, 1)].squeeze(3),
                        op=OP.add,
                    )
                    nc.scalar.activation(sc["sif"][:], sc["g"][:, 0:4, :], AF.Sigmoid)
                    nc.scalar.activation(sc["tg"][:], sc["g"][:, 4:6, :], AF.Tanh)
                    nc.scalar.activation(sc["so"][:], sc["g"][:, 6:8, :], AF.Sigmoid)
                    nc.vector.tensor_tensor(out=sc["t1"][:], in0=sc["sif"][:, 0:2, :],
                                            in1=sc["tg"][:], op=OP.mult)
                    nc.vector.tensor_tensor(out=sc["c"][:], in0=sc["sif"][:, 2:4, :],
                                            in1=sc["c"][:], op=OP.mult)
                    nc.vector.tensor_tensor(out=sc["c"][:], in0=sc["c"][:],
                                            in1=sc["t1"][:], op=OP.add)
                    nc.scalar.activation(sc["tc"][:], sc["c"][:], AF.Tanh)
                    nc.vector.tensor_tensor(
                        out=hst[:, :, :, ds(wr, 1)].squeeze(3),
                        in0=sc["so"][:], in1=sc["tc"][:], op=OP.mult,
                    )

            if run_lstm:
                tc.For_i_unrolled(0, s, 1, lstm_step, max_unroll=8)
            else:
                nc.vector.memset(hf[:], 0.0)
                nc.vector.memset(hb[:], 0.0)

            # ================= P4: CRF init (t=0) ==============================
            ps_ft = ps_tr if crf_db else ps_c
            ps_q = ps_pj if crf_db else ps_c

            def feat_mm(t_fwd_slot, t_bwd_slot):
                ftp = ps_ft.tile([T, BL], F32, tag="ft" if not crf_db else "tr")
                for kk in range(4):
                    hst, slot = (hf, t_fwd_slot) if kk < 2 else (hb, t_bwd_slot)
                    nc.tensor.matmul(
                        ftp[:],
                        lhsT=wtag_sb[:, kk, :],
                        rhs=hst[:, kk % 2, :, ds(slot, 1)].squeeze(2),
                        start=(kk == 0),
                        stop=(kk == 3),
                    )
                return ftp

            prev = pp.tile([32, 32], F32)
            pex = pp.tile([32, 32], F32)
            pexT = pp.tile([32, 32], F32)
            candT = pp.tile([32, 32], F32)
            candA = pp.tile([32, 32], F32)
            dd = pp.tile([32, 32], F32)
            nm = pp.tile([BL, 1], F32)
            nc.vector.memset(prev[:], 0.0)
            nc.vector.memset(pex[:], 0.0)
            nc.vector.memset(candT[:], 0.0)

            ftp0 = feat_mm(1, 0)
            nc.vector.tensor_copy(feats_sb[0:T, :, 0], ftp0[:])
            nc.vector.tensor_scalar_add(candT[0:T, 0:BL], ftp0[:], stv_sb[:])
            nc.vector.transpose(prev[:], candT[:])

            # ================= P5: CRF forward loop (t = 1..s-1) ===============
            def crf_step(t):
                ftp = feat_mm(t + 1, t)
                nc.vector.tensor_copy(feats_sb[0:T, :, ds(t, 1)].squeeze(2), ftp[:])
                nc.vector.tensor_reduce(
                    nm[:], prev[0:BL, 0:T], axis=AX.X, op=OP.max, negate=True
                )
                nc.scalar.activation(pex[0:BL, 0:T], prev[0:BL, 0:T], AF.Exp,
                                     bias=nm[:])
                nc.vector.transpose(pexT[:], pex[:])
                qp = ps_q.tile([T, BL], F32, tag="q" if not crf_db else "pj")
                nc.tensor.matmul(qp[:], lhsT=e_sb[:], rhs=pexT[0:T, 0:BL],
                                 start=True, stop=True)
                nc.scalar.activation(candT[0:T, 0:BL], qp[:], AF.Ln)
                nc.vector.tensor_tensor(out=candT[0:T, 0:BL], in0=candT[0:T, 0:BL],
                                        in1=ftp[:], op=OP.add)
                nc.vector.transpose(candA[:], candT[:])
                nc.vector.scalar_tensor_tensor(
                    out=dd[0:BL, 0:T], in0=candA[0:BL, 0:T], scalar=nm[:],
                    in1=prev[0:BL, 0:T], op0=OP.subtract, op1=OP.subtract,
                )
                nc.vector.scalar_tensor_tensor(
                    out=prev[0:BL, 0:T], in0=dd[0:BL, 0:T],
                    scalar=mask_sb[:, ds(t, 1)],
                    in1=prev[0:BL, 0:T], op0=OP.mult, op1=OP.add,
                )

            if run_crf:
                tc.For_i_unrolled(1, s, 1, crf_step, max_unroll=crf_unroll)

            # ================= P6: forward score + outputs =====================
            ssum = pp.tile([BL, 1], F32)
            fs0 = pp.tile([BL, 1], F32)
            fs = pp.tile([BL, 1], F32)
            nc.vector.tensor_reduce(nm[:], prev[0:BL, 0:T], axis=AX.X, op=OP.max,
                                    negate=True)
            nc.scalar.activation(pex[0:BL, 0:T], prev[0:BL, 0:T], AF.Exp,
                                 bias=nm[:], accum_out=ssum[:])
            nc.scalar.activation(fs0[:], ssum[:], AF.Ln)
            nc.vector.tensor_scalar_sub(fs[:], fs0[:], nm[:])
            nc.sync.dma_start(fs_out[:], fs[:])

            # ========== P7: gold emit score on device ==========
            # emit_sum[b] = sum_t feats[label[b,t], b, t] for t < len
            # labf holds label (float) or -1 for padded steps -> no tag match.
            ones1 = pp.tile([1, T], F32)
            nc.vector.memset(ones1[:], 1.0)
            onesT = pp.tile([T, 1], F32)
            nc.vector.memset(onesT[:], 1.0)
            emtag = pp.tile([T, BL], F32)
            ohp = pp.tile([T, s], F32)
            for b in range(BL):
                lbp = ps_pj.tile([T, s], F32, tag="pj")
                nc.tensor.matmul(lbp[:], lhsT=ones1[:], rhs=labf_sb[:, b, :],
                                 start=True, stop=True)
                nc.vector.tensor_scalar(out=ohp[:], in0=lbp[:], scalar1=tgi_sb[:],
                                        scalar2=None, op0=OP.is_equal)
                nc.vector.tensor_tensor(out=ohp[:], in0=ohp[:],
                                        in1=feats_sb[0:T, b, :], op=OP.mult)
                nc.vector.tensor_reduce(emtag[:, b : b + 1], ohp[:], axis=AX.X,
                                        op=OP.add)
            emp = ps_c.tile([1, BL], F32, tag="ft" if not crf_db else "tr")
            nc.tensor.matmul(emp[:], lhsT=onesT[:], rhs=emtag[:],
                             start=True, stop=True)
            em = pp.tile([1, BL], F32)
            nc.vector.tensor_copy(em[:], emp[:])
            nc.sync.dma_start(em_out[:], em[:])

    nc.compile()
    return nc


# ======================= host-side persistent runner =======================


def prep_consts(embedding, w_ih_f, w_hh_f, b_f, w_ih_b, w_hh_b, b_b,
                w_tag, b_tag, transitions):
    """Host-side packing of the replicated constants (per-core shapes)."""
    emb = np.ascontiguousarray(np.asarray(embedding, np.float32))

    wcat = np.zeros((EP, G), np.float32)
    wcat[:E, : 4 * HD] = np.asarray(w_ih_f, np.float32).T
    wcat[:E, 4 * HD :] = np.asarray(w_ih_b, np.float32).T
    wcat[E, : 4 * HD] = np.asarray(b_f, np.float32)
    wcat[E, 4 * HD :] = np.asarray(b_b, np.float32)

    whh = np.zeros((HD, G), np.float32)
    whh[:, : 4 * HD] = np.asarray(w_hh_f, np.float32).T
    whh[:, 4 * HD :] = np.asarray(w_hh_b, np.float32).T

    wtag = np.zeros((4 * 128, T), np.float32)
    wtag[: 2 * HD] = np.asarray(w_tag, np.float32).T

    trans = np.asarray(transitions, np.float32)
    btag = np.asarray(b_tag, np.float32)
    ecrf = np.exp(np.maximum(trans, TRANS_CLIP) + btag[None, :]).astype(np.float32)
    stvec = (trans[START_TAG] + btag).astype(np.float32).reshape(T, 1)

    return {
        "emb": emb,
        "wcat": wcat.astype(ml_dtypes.bfloat16),
        "whh": whh.astype(ml_dtypes.bfloat16),
        "wtag": wtag.astype(ml_dtypes.bfloat16),
        "ecrf": ecrf,
        "stv": stvec,
        "tgi": np.arange(T, dtype=np.float32).reshape(T, 1),
    }


def prep_percall(data, text_lengths, label):
    """Per-call inputs, concatenated across cores along axis 0."""
    data = np.asarray(data)
    lengths = np.asarray(text_lengths)
    label = np.asarray(label)
    msk = np.arange(S)[None, :] < lengths[:, None]              # [B, S]
    labf = np.where(msk, label, -1).astype(np.float32)          # -1: no tag match
    mask_f = msk.astype(np.float32)
    toks = []
    for c in range(NCORES):
        flat = data[c * BL : (c + 1) * BL].reshape(-1).astype(np.int32)
        toks.append(flat.reshape(-1, 128).T.copy())       # tok[p, i] = flat[i*128+p]
    return {
        "tok": np.concatenate(toks, axis=0),
        "mask": mask_f,
        "labf": labf.reshape(NCORES, BL, S),  # [1, BL, S] per core
    }


class Runner:
    """Persistent jitted shard_map executable with device-cached constants."""

    def __init__(self, nc):
        import jax
        from jax.experimental.shard_map import shard_map
        from jax.sharding import Mesh, PartitionSpec, NamedSharding
        from concourse.bass2jax import (install_neuronx_cc_hook, _bass_exec_p,
                                        partition_id_tensor)

        install_neuronx_cc_hook()
        assert nc.dbg_addr is None

        self.jax = jax
        partition_name = (nc.partition_id_tensor.name
                          if nc.partition_id_tensor else None)
        in_names, out_names, out_avals = [], [], []
        for alloc in nc.m.functions[0].allocations:
            if not isinstance(alloc, mybir.MemoryLocationSet):
                continue
            name = alloc.memorylocations[0].name
            if alloc.kind == "ExternalInput":
                if name != partition_name:
                    in_names.append(name)
            elif alloc.kind == "ExternalOutput":
                out_names.append(name)
                out_avals.append(
                    jax.core.ShapedArray(tuple(alloc.tensor_shape),
                                         mybir.dt.np(alloc.dtype))
                )
        self.in_names, self.out_names, self.out_avals = in_names, out_names, out_avals
        n_params = len(in_names)
        n_outs = len(out_names)

        all_in_names = in_names + out_names
        if partition_name is not None:
            all_in_names = all_in_names + [partition_name]

        def _body(*args):
            operands = list(args)
            if partition_name is not None:
                operands.append(partition_id_tensor())
            outs = _bass_exec_p.bind(
                *operands,
                out_avals=tuple(out_avals),
                in_names=tuple(all_in_names),
                out_names=tuple(out_names),
                lowering_input_output_aliases=(),
                sim_require_finite=True,
                sim_require_nnan=True,
                nc=nc,
            )
            return tuple(outs)

        devices = jax.devices()[:NCORES]
        self.mesh = Mesh(np.asarray(devices), ("core",))
        pspec = PartitionSpec("core")
        self.sharding = NamedSharding(self.mesh, pspec)
        in_specs = (pspec,) * (n_params + n_outs)
        out_specs = (pspec,) * n_outs
        self.fn = jax.jit(
            shard_map(_body, mesh=self.mesh, in_specs=in_specs,
                      out_specs=out_specs, check_rep=False),
            donate_argnums=tuple(range(n_params, n_params + n_outs)),
            keep_unused=True,
        )
        self.const_dev = {}
        self._out_bufs = None

    def put_replicated(self, consts):
        """Commit per-core-replicated constants to all 8 devices (once)."""
        for name, arr in consts.items():
            gshape = (NCORES * arr.shape[0],) + arr.shape[1:]
            self.const_dev[name] = self.jax.make_array_from_callback(
                gshape, self.sharding, lambda idx, a=arr: a
            )

    def __call__(self, percall):
        args = [percall.get(n, self.const_dev.get(n)) for n in self.in_names]
        assert all(a is not None for a in args), "missing input"
        # Donated output buffers: recycle last call's outputs (the kernel
        # overwrites every element, so contents don't matter); avoids a fresh
        # host->device transfer of zero buffers each call. Pre-commit the
        # initial zeros with the same sharding the recycled outputs will
        # carry, so every call hits the same jitted executable.
        bufs = self._out_bufs
        if bufs is None:
            bufs = [self.jax.device_put(
                        np.zeros((NCORES * a.shape[0],) + tuple(a.shape[1:]),
                                 a.dtype), self.sharding)
                    for a in self.out_avals]
        outs = self.fn(*args, *bufs)
        res = {
            n: np.asarray(outs[i]).reshape((NCORES,) + tuple(self.out_avals[i].shape))
            for i, n in enumerate(self.out_names)
        }
        self._out_bufs = list(outs)
        return res


_STATE = {}


def _get_nc():
    if "nc" not in _STATE:
        _STATE["nc"] = build(S)
    return _STATE["nc"]


def _get_runner():
    """Persistent fast path; falls back to run_bass_kernel_spmd if the
    hoisted jit/shard_map path cannot be constructed in this environment."""
    if "runner" not in _STATE:
        try:
            _STATE["runner"] = Runner(_get_nc())
        except Exception:
            _STATE["runner"] = None
    return _STATE["runner"]


def _run_fallback(percall, consts):
    from concourse.bass_utils import run_bass_kernel_spmd
    nc = _get_nc()
    in_maps = []
    for c in range(NCORES):
        m = {k: v[c * v.shape[0] // NCORES : (c + 1) * v.shape[0] // NCORES]
             for k, v in percall.items()}
        m.update(consts)
        in_maps.append(m)
    res = run_bass_kernel_spmd(nc, in_maps, core_ids=list(range(NCORES)))
    return {
        name: np.stack([r[name] for r in res.results])
        for name in ("em_out", "fs_out")
    }


def kernel(data, label, text_lengths, embedding, w_ih_f, w_hh_f, b_f,
           w_ih_b, w_hh_b, b_b, w_tag, b_tag, transitions):
    runner = _get_runner()

    # device-side constants: commit once, re-commit only if they change
    raw_consts = tuple(np.asarray(a) for a in
                       (embedding, w_ih_f, w_hh_f, b_f, w_ih_b, w_hh_b, b_b,
                        w_tag, b_tag, transitions))
    refs = _STATE.get("const_refs")
    copies = _STATE.get("const_copies")
    if refs is not None and all(a is b for a, b in zip(raw_consts, refs)):
        pass  # same array objects: fast path
    elif copies is not None and all(
        np.array_equal(a, b) for a, b in zip(raw_consts, copies)
    ):
        _STATE["const_refs"] = raw_consts
    else:
        _STATE["const_refs"] = raw_consts
        _STATE["const_copies"] = tuple(np.array(a, copy=True) for a in raw_consts)
        _STATE["consts"] = prep_consts(*raw_consts)
        if runner is not None:
            runner.put_replicated(_STATE["consts"])

    percall = prep_percall(data, text_lengths, label)
    if runner is not None:
        out = runner(percall)
    else:
        out = _run_fallback(percall, _STATE["consts"])

    forward_score = out["fs_out"].astype(np.float32).reshape(B)
    emit_dev = out["em_out"].astype(np.float32).reshape(B)  # no b_tag

    # ---- rest of the gold score on host (indexing only) ----
    label = np.asarray(label)
    lengths = np.asarray(text_lengths)
    trans = np.asarray(transitions, np.float32)
    btag = np.asarray(b_tag, np.float32)
    msk = (np.arange(S)[None, :] < lengths[:, None]).astype(np.float32)
    emit_sum = emit_dev + np.sum(btag[label] * msk, axis=1)
    tr_pair = trans[label[:, :-1], label[:, 1:]]
    tr_sum = np.sum(tr_pair * msk[:, 1:], axis=1)
    start_tr = trans[START_TAG, label[:, 0]]
    last_tag = label[np.arange(B), lengths - 1]
    stop_tr = trans[last_tag, STOP_TAG]
    gold = emit_sum + tr_sum + start_tr + stop_tr

    loss = np.sum(forward_score - gold) / B
    return np.float32(loss)
